# revision 1
# baseline (speedup 1.0000x reference)
"""CRF log-partition kernel for Trainium2 (8 NeuronCores, data-parallel batch).

Algorithm: the reference forward scan
    alpha' = logsumexp(alpha[None,:] + trans, axis=prev) + emit
is linearized to probability space:
    p' = (M @ p) * E,   M = exp(trans), E = exp(emit) * 2^-7
(the 2^-7 cancels the mean per-step log-growth of ~4.85, so the state
stays in f32/bf16 range with no renormalization; all scale bookkeeping
is recovered on the host from state snapshots).

Each batch item's 2048-step sequence is split into P=32 segments of L=64
steps scanned in parallel (products of positive matrices converge to
rank-1, so each segment's output direction is independent of its init;
scales are fixed up by an 8-tick prefix-correction pass seeded with the
previous segment's final state). Per core: 32 batch x 32 segments = 1024
chains laid out as X[128, 512] bf16 — tag-block A (chains 0-511) on
partitions 0-47, block B (chains 512-1023) on partitions 64-111 (engine
APs require partition bases 0/32/64/96; junk rows are written but killed
by zero rows of the padded [112,128] lhsT). Per tick: two half-column
matmuls + DVE multiplies ping-pong so independent chains hide the
PE<->DVE semaphore latency. Emissions stream in natural layout
(contiguous per-partition DMA on both HWDGE rings), are exp'd on ACT
into resident bf16 tiles, and transposed just-in-time to PSUM by the PE
(widened 64-row reads fully cover the tile); one ACT/DVE copy per 4-step
quad moves E to SBUF (HW allows one PSUM operand per vector op).
Cost-model estimate ~90us/core vs ~35us memory roofline.

Host stitches per-(batch,segment) log-scales in float64 from three bf16
snapshots per core: pass-1 state at tick 16, pass-1 final, pass-2 final.

mask does not affect the forward value (m*x + (1-m)*x == x) and is ignored.
"""

import math

import numpy as np

B, S, T = 256, 2048, 48
NEG = -10000.0
NCORES = 8
BC = B // NCORES          # batch per core = 32
P = 32                    # segments per batch item
L = S // P                # ticks per segment = 64
PREFIX = 8                # prefix-correction ticks
C2POW = -7                # constant rescale folded into E
COLS = 512                # chains per block (columns of X)
ROWS = 112                # meaningful partition rows (blocks at 0-47 / 64-111)
XROWS = 128               # physical tile rows (junk rows fully written)
TW = 64                   # transpose read-width (48 tags + 16 junk cols)
NGRP = 8                  # DMA chain-groups of 128 chains
WSIZES = (16, 16, 16, 16)  # E window sizes (sum = L)
RING_SPLIT = 1            # issue alternate window-0 loads on the ACT HWDGE ring
ETBUFS = 2                # PSUM et tiles in flight
NHALF = 2                 # matmul/multiply column-split for latency hiding
XBUFS = 3
ESBUFS = 3
QBUFS = 4
ETSTEPS = 4               # steps per et tile (2=pair, 4=quad)
DVE_COPY = 4              # of every 16 et copies, this many go to DVE

_CACHE = {}


def _build(**cfg):
    g = globals()
    saved = {k: g[k] for k in cfg}
    g.update(cfg)
    try:
        return _build_inner()
    finally:
        g.update(saved)


def _build_inner():
    from contextlib import ExitStack

    import concourse.bacc as bacc
    import concourse.bass as bass
    import concourse.mybir as mybir
    import concourse.tile as tile

    f32 = mybir.dt.float32
    bf16 = mybir.dt.bfloat16

    nc = bacc.Bacc(None, target_bir_lowering=False)

    em_d = nc.dram_tensor("emissions", [BC, S, T], f32, kind="ExternalInput")
    w_d = nc.dram_tensor("wlhs", [ROWS, XROWS], bf16, kind="ExternalInput")
    pinit_d = nc.dram_tensor("pinit", [48, 32], bf16, kind="ExternalInput")
    ident_d = nc.dram_tensor("ident", [128, 128], bf16, kind="ExternalInput")
    snap16_d = nc.dram_tensor("snap16", [ROWS, COLS], bf16, kind="ExternalOutput")
    snapf_d = nc.dram_tensor("snapf", [ROWS, COLS], bf16, kind="ExternalOutput")
    snap2_d = nc.dram_tensor("snap2", [ROWS, COLS], bf16, kind="ExternalOutput")

    EXP_BIAS = float(C2POW * math.log(2.0))
    assert sum(WSIZES) == L
    WOFF = [sum(WSIZES[:i]) for i in range(len(WSIZES))]
    kmap = {}  # absolute step -> (window, step-within-window)
    for wi, ws in enumerate(WSIZES):
        for kk in range(ws):
            kmap[WOFF[wi] + kk] = (wi, kk)

    with tile.TileContext(nc) as tc:
        with ExitStack() as ctx:
            consts = ctx.enter_context(tc.tile_pool(name="consts", bufs=1))
            stage = ctx.enter_context(tc.tile_pool(name="stage", bufs=3))
            HALF = COLS // NHALF
            epool = ctx.enter_context(tc.tile_pool(name="epool", bufs=NGRP * len(WSIZES)))
            xpool = ctx.enter_context(tc.tile_pool(name="xpool", bufs=XBUFS))
            qpool = ctx.enter_context(
                tc.tile_pool(name="qpool", bufs=QBUFS, space=bass.MemorySpace.PSUM))
            espool = ctx.enter_context(tc.tile_pool(name="espool", bufs=ESBUFS))
            esp0 = ctx.enter_context(
                tc.tile_pool(name="esp0", bufs=max(1, PREFIX // ETSTEPS)))
            etpool = ctx.enter_context(
                tc.tile_pool(name="etpool", bufs=ETBUFS, space=bass.MemorySpace.PSUM))

            w_sb = consts.tile([ROWS, XROWS], bf16, tag="w")
            nc.sync.dma_start(w_sb[:], w_d[:])
            ident_sb = consts.tile([128, 128], bf16, tag="ident")
            nc.sync.dma_start(ident_sb[:], ident_d[:])
            bias_sb = consts.tile([128, 1], f32, tag="bias")
            nc.gpsimd.memset(bias_sb[:], EXP_BIAS)

            # emissions with each partition holding one chain's segment:
            # [b, s, t] -> [seg, b, f(= L steps x 48 tags, contiguous)]
            em_r = em_d[:].rearrange("b s t -> b (s t)") \
                          .rearrange("b (seg f) -> seg b f", seg=P)

            # Load + exp, window-major with small early windows so the scan
            # starts as soon as the first 4 steps of E are resident.
            etiles = [[None] * len(WSIZES) for _ in range(NGRP)]
            for wi, ws in enumerate(WSIZES):
                fa, fb = WOFF[wi] * T, (WOFF[wi] + ws) * T
                for g in range(NGRP):
                    st = stage.tile([128, fb - fa], f32, tag=f"stage{ws}",
                                    name=f"st{wi}_{g}")
                    dma_eng = nc.scalar if (RING_SPLIT and g % 2
                                            and wi < RING_SPLIT) else nc.sync
                    dma_eng.dma_start(st[:], em_r[4 * g:4 * g + 4, :, fa:fb])
                    # ([4,32,F] -> [128,F]: same element iteration order,
                    # partition j = 32*sub + b)
                    et = epool.tile([128, fb - fa + TW - T], bf16,
                                    tag=f"e{ws}", name=f"e{wi}_{g}")
                    nc.scalar.activation(
                        et[:, 0:fb - fa], st[:],
                        mybir.ActivationFunctionType.Exp, bias=bias_sb[:])
                    nc.gpsimd.memset(et[:, fb - fa:], 1.0)  # widened-read pad
                    etiles[g][wi] = et

            # init X: ones everywhere, true-init vector in block A cols 0-31
            x = xpool.tile([XROWS, COLS], bf16, tag="x")
            nc.gpsimd.memset(x[:], 1.0)
            nc.sync.dma_start(x[0:48, 0:32], pinit_d[:])

            def make_et_group(k0):
                """Steps k0..k0+ETSTEPS-1, all 8 groups -> one SBUF tile.

                PE transposes 64-wide slices (48 tags + 16 junk; junk rows are
                killed by zero rows of the padded lhsT) into a PSUM tile,
                fully covering it; one ACT (or DVE) copy moves it to SBUF (HW
                allows only one PSUM operand per vector op, so the multiply
                needs E in SBUF).
                """
                etp = etpool.tile([XROWS, ETSTEPS * COLS], bf16, tag="et")
                for s in range(ETSTEPS):
                    for g in range(NGRP):
                        w, kk = kmap[k0 + s]
                        src = etiles[g][w][:, kk * T:kk * T + TW]   # [128, 64]
                        dst = etp[64 * (g // 4):64 * (g // 4) + 64,
                                  s * COLS + 128 * (g % 4):
                                  s * COLS + 128 * (g % 4) + 128]   # [64, 128]
                        nc.tensor.transpose(dst, src, ident_sb[:])
                # window-0 tiles persist in their own pool: pass 2 reuses
                # them directly (no transposes/copies in pass 2)
                pool = esp0 if k0 < PREFIX else espool
                et_sb = pool.tile([XROWS, ETSTEPS * COLS], bf16, tag="es",
                                  name=f"es{k0}")
                if (k0 // ETSTEPS) % 16 < DVE_COPY:
                    nc.vector.tensor_copy(et_sb[:], etp[:])
                else:
                    nc.scalar.activation(
                        et_sb[:], etp[:], mybir.ActivationFunctionType.Copy)
                return et_sb

            def tick(x_in, et_sl, matmul):
                x_out = xpool.tile([XROWS, COLS], bf16, tag="x")
                if matmul:
                    # column-halves in separate PSUM tiles (banks) so the
                    # PE->DVE chain pipelines across independent chains
                    for h in range(NHALF):
                        cs = slice(h * HALF, (h + 1) * HALF)
                        q = qpool.tile([XROWS, HALF], f32, tag="q",
                                       name=f"q{h}")
                        nc.tensor.matmul(q[:], w_sb[:], x_in[0:ROWS, cs])
                        nc.vector.tensor_mul(x_out[:, cs], q[:], et_sl[:, cs])
                else:
                    nc.vector.tensor_mul(x_out[:], x_in[:], et_sl[:])
                return x_out

            # pass 1
            et_sb = None
            es0 = []
            for k in range(L):
                if k % ETSTEPS == 0:
                    et_sb = make_et_group(k)
                    if k < PREFIX:
                        es0.append(et_sb)
                sl = et_sb[:, (k % ETSTEPS) * COLS:(k % ETSTEPS + 1) * COLS]
                x = tick(x, sl, matmul=(k > 0))
                if k + 1 == PREFIX:
                    nc.sync.dma_start(snap16_d[:], x[0:ROWS, :])
            nc.sync.dma_start(snapf_d[:], x[0:ROWS, :])

            # pass 2: init = pass-1 finals shifted by one segment slot
            x2 = xpool.tile([XROWS, COLS], bf16, tag="x")
            nc.gpsimd.memset(x2[:, 0:32], 1.0)                    # seg-0 slot unused
            nc.vector.tensor_copy(x2[:, 32:COLS], x[:, 0:COLS - 32])
            nc.sync.dma_start(x2[64:112, 0:32], x[0:48, COLS - 32:COLS])
            for k in range(PREFIX):
                sl = es0[k // ETSTEPS][:, (k % ETSTEPS) * COLS:
                                       (k % ETSTEPS + 1) * COLS]
                x2 = tick(x2, sl, matmul=True)
            nc.sync.dma_start(snap2_d[:], x2[0:ROWS, :])

    nc.compile()
    return nc


def _host_consts(transitions):
    """W lhsT, p_init (analytic first log-step), identity, stitch constants."""
    import ml_dtypes

    tr = transitions.astype(np.float64)
    M = np.exp(tr)                                   # M[next, prev]
    wl = np.zeros((ROWS, XROWS), np.float64)
    wl[0:48, 0:48] = M.T                             # lhsT[k, m] = M[m, k]
    wl[64:112, 64:112] = M.T

    # analytic first step: v[next] = logsumexp_prev(tr[next, :] + alpha0)
    alpha0 = np.full(T, NEG, np.float64)
    alpha0[0] = 0.0
    sc = tr + alpha0[None, :]
    mm = sc.max(axis=1, keepdims=True)
    v = np.log(np.exp(sc - mm).sum(axis=1)) + mm[:, 0]
    vmax = v.max()
    p_init = np.exp(v - vmax)                        # [T]

    bf = ml_dtypes.bfloat16
    w_np = wl.astype(bf)
    pinit_np = np.repeat(p_init[:, None], 32, axis=1).astype(bf)
    ident_np = np.eye(128, dtype=np.float64).astype(bf)

    r = tr[-1, :]
    r_max = r.max()
    w_last = np.exp(r - r_max)                       # final-row weights [T]
    return w_np, pinit_np, ident_np, vmax, r_max, w_last


def _stitch(snap16, snapf, snap2, vmax, r_max, w_last):
    """Per-core host stitch -> [BC] log partition (float64)."""
    def tags(a):  # [112, COLS] -> [T, P, BC] per-chain tag values
        a = np.asarray(a, np.float64)
        return np.concatenate([a[0:48, :], a[64:112, :]], axis=1) \
                 .reshape(T, P, BC)                   # chain = seg*BC + b

    s16 = np.log(np.maximum(tags(snap16).sum(axis=0), 1e-300))   # [P, BC]
    last = tags(snapf)
    sf = np.log(np.maximum(last.sum(axis=0), 1e-300))
    s2 = np.log(np.maximum(tags(snap2).sum(axis=0), 1e-300))

    Lfin = sf[P - 1, :] + (s2[1:, :] - s16[1:, :]).sum(axis=0)

    fin = last[:, -1, :]                              # [T, BC] final-seg state
    d = np.log(np.maximum((w_last[:, None] * fin).sum(axis=0), 1e-300)) \
        - np.log(np.maximum(fin.sum(axis=0), 1e-300))

    return Lfin + d + r_max + vmax - S * C2POW * math.log(2.0)


def kernel(**inputs):
    emissions = np.ascontiguousarray(inputs["emissions"], dtype=np.float32)
    transitions = np.asarray(inputs["transitions"], dtype=np.float32)

    if "nc" not in _CACHE:
        _CACHE["nc"] = _build()
    nc = _CACHE["nc"]

    w_np, pinit_np, ident_np, vmax, r_max, w_last = _host_consts(transitions)

    in_maps = []
    for c in range(NCORES):
        in_maps.append({
            "emissions": emissions[c * BC:(c + 1) * BC],
            "wlhs": w_np,
            "pinit": pinit_np,
            "ident": ident_np,
        })

    from concourse.bass_utils import run_bass_kernel_spmd
    res = run_bass_kernel_spmd(nc, in_maps, list(range(NCORES))).results

    out = np.empty(B, np.float32)
    for c in range(NCORES):
        r = res[c]
        out[c * BC:(c + 1) * BC] = _stitch(
            r["snap16"], r["snapf"], r["snap2"], vmax, r_max, w_last
        ).astype(np.float32)
    return out

